# revision 22
# baseline (speedup 1.0000x reference)
"""Trainium2 Bass kernel for nn_ControlModel_g (phi^4 lattice control-variate loss).

Math reformulation (validated to fp32 accuracy against the jax reference):

  The reference evaluates, for each of 16 signed lattice symmetries t and all
  V=256 torus translations s, the tiny MLP g (256->128->1) on the transformed+
  shifted configs, plus its input-gradient at site (0,0), combined with the
  phi^4 force into F[b]; loss = mean((computeO(x) - F - muO)^2).

  1. Symmetry transforms move from x onto W1 (g(T_{-s} R x) = g_R(T_{-s'} x)
     with spatially-transformed weights), so all shifted inputs derive from x
     alone and the force/gradient corrections become fixed permutations.
  2. With b1 == 0 (always true for this model), tanh oddness makes the 8
     sign=-1 transforms algebraically redundant -> half the compute.
  3. The column translation j is moved onto the DATA side: instead of 16
     j-rotated copies of W1 per core, each core gets its two j-rotations of
     the shifted-x matrix SHJ_j[(a,c),(i,b)] = x[b,(a+i)%16,(c+j)%16] and the
     8 spatially-permuted W1_r weight images are shared across j. Device work
     per core: 16 group matmuls Z_{r,jl} = W1_r^T @ SHJ_jl (f32 accum over
     the 256-site contraction in 2 psum passes), tanh (scalar ACT), square
     (DVE), and two 1-row PE reductions per group with a shared [W2|CW]
     stationary (tile_position column strips, 4 groups per PSUM set).
  4. All operands are bf16 (error budget validated: ~6e-4 rel on the loss vs
     2e-2 tolerance); psum accumulation stays fp32.
  5. Sharding: data-parallel over the j columns - core cc takes j in
     {2cc, 2cc+1}. No collectives; the final O(B*V) combine (force
     permutations, computeO, loss) is host-side numpy.
"""

import numpy as np
import ml_dtypes

L = 16
Y = 4
KAPPA = 0.25
LAM = 0.5
B = 32
V = L * L          # 256
H = 128
NCORES = 8
JPER = L // NCORES         # j values per core = 2
M = L * B                  # 512 columns (i, b) per group
NSETS = 4                  # r-pairs; each set = 4 groups (rl, jl)

# ---------------------------------------------------------------------------
# host-side lattice helpers
# ---------------------------------------------------------------------------

def _force(phi):
    nbr = (np.roll(phi, 1, 1) + np.roll(phi, -1, 1)
           + np.roll(phi, 1, 2) + np.roll(phi, -1, 2))
    return 2.0 * KAPPA * nbr - 2.0 * phi - 4.0 * LAM * phi * (phi * phi - 1.0)


def _computeO(x):
    x0 = x.mean(axis=1)
    x0 = x0 - x0.mean(axis=0, keepdims=True)
    return (x0 * np.roll(x0, -Y, axis=1)).mean(axis=1)


def _spatial_ops():
    ops = []
    for k in range(4):
        ops.append(lambda y, k=k: np.rot90(y, k=k, axes=(0, 1)))
        ops.append(lambda y, k=k: np.flip(np.rot90(y, k=k, axes=(0, 1)), axis=0))
    return ops


def _op_tables():
    """Per spatial op r: inverse site permutation (for W1) and the force
    permutation mu_r[s] = pi_r(rho_r^{-1}(s))."""
    ops = _spatial_ops()
    IDX = np.arange(V).reshape(L, L)
    inv_perms, mus = [], []
    for op in ops:
        pi = op(IDX).reshape(-1)
        inv = np.empty(V, np.int64)
        inv[pi] = np.arange(V)
        inv_perms.append(inv)
        rho = np.empty(V, np.int64)
        opIDX = op(IDX)
        for i in range(L):
            for j in range(L):
                shifted = np.roll(np.roll(opIDX, -i, 0), -j, 1).reshape(V)
                rho[i * L + j] = shifted[inv][0]
        rho_inv = np.empty(V, np.int64)
        rho_inv[rho] = np.arange(V)
        mus.append(pi[rho_inv])
    return inv_perms, mus


_TABLES = None

def _tables():
    global _TABLES
    if _TABLES is None:
        _TABLES = _op_tables()
    return _TABLES


# ---------------------------------------------------------------------------
# device program (built once, cached)
# ---------------------------------------------------------------------------

_PROG = None

def _build_program():
    import concourse.bass as bass
    import concourse.tile as tile
    from concourse import bacc, mybir

    f32 = mybir.dt.float32
    bf16 = mybir.dt.bfloat16
    MUL = mybir.AluOpType.mult

    nc = bacc.Bacc("TRN2", target_bir_lowering=False, debug=False,
                   num_devices=NCORES)

    # packed inputs: few big fully-contiguous DMAs (each dma_start costs
    # ~600ns of queue time; column-sliced HBM reads run at half bandwidth)
    # shj_k cols: (jl, m); w1r_k cols: (r, h)
    shj_ds = [nc.dram_tensor(f"shj{k}", (128, 2 * M), bf16,
                             kind="ExternalInput") for k in range(2)]
    w1r_ds = [nc.dram_tensor(f"w1r{k}", (128, 8 * H), bf16,
                             kind="ExternalInput") for k in range(2)]
    rw_d = nc.dram_tensor("rw", (128, 2), bf16, kind="ExternalInput")
    out_d = nc.dram_tensor("gvgd", (NSETS, 4, 2, 2 * M), f32,
                           kind="ExternalOutput")

    NWARM = 9

    with tile.TileContext(nc) as tc:
        with (
            tc.tile_pool(name="consts", bufs=1) as cpool,
            tc.tile_pool(name="zp", bufs=3, space=bass.MemorySpace.PSUM) as zpool,
            tc.tile_pool(name="rp", bufs=1, space=bass.MemorySpace.PSUM) as rpool,
            tc.tile_pool(name="work", bufs=2) as wpool,
        ):
            # ---- PE warmup: keep the HAM clock-gate busy during input DMA -
            if NWARM:
                ws = cpool.tile([128, M + 128], bf16, tag="ws")
                nc.vector.memset(ws[:], 0.0)
                wz = zpool.tile([128, 2 * M], f32, tag="zt")
                for _ in range(NWARM):
                    nc.tensor.matmul(wz[:, 0:M], ws[:, 0:128], ws[:, 128:],
                                     start=True, stop=True)

            # ---- input DMAs: 5 contiguous transfers on two idle queues ----
            # shj first (gates the first matmuls); rw last (needed ~10us in)
            shj_t, w1r_t = [], []
            for k in range(2):
                sht = cpool.tile([128, 2 * M], bf16, tag=f"shj{k}")
                nc.sync.dma_start(sht[:], shj_ds[k][:])
                shj_t.append(sht)
                w1t = cpool.tile([128, 8 * H], bf16, tag=f"w1r{k}")
                nc.gpsimd.dma_start(w1t[:], w1r_ds[k][:])
                w1r_t.append(w1t)
            rw_t = cpool.tile([128, 2], bf16, tag="rw")
            nc.sync.dma_start(rw_t[:], rw_d[:])

            def shj_blk(jl, k):
                return shj_t[k][:, jl * M:(jl + 1) * M]

            def w1r_blk(r, k):
                return w1r_t[k][:, r * H:(r + 1) * H]

            # ---- pipelined compute ---------------------------------------
            at_s = [None] * NSETS
            asq_s = [None] * NSETS

            def emit_main(s, mid=None):
                at = wpool.tile([128, 4 * M], bf16, tag="at")
                asq = wpool.tile([128, 4 * M], bf16, tag="asq")
                at_s[s], asq_s[s] = at, asq
                for rl in range(2):
                    if rl == 1 and mid is not None:
                        mid()
                    r = 2 * s + rl
                    zt = zpool.tile([128, 2 * M], f32, tag="zt")
                    for k in range(2):
                        for jl in range(2):
                            nc.tensor.matmul(
                                zt[:, jl * M:(jl + 1) * M],
                                w1r_blk(r, k),
                                shj_blk(jl, k),
                                start=(k == 0),
                                stop=(k == 1),
                            )
                    hs = slice(rl * 2 * M, (rl + 1) * 2 * M)
                    nc.scalar.activation(at[:, hs], zt[:],
                                         mybir.ActivationFunctionType.Tanh)
                    nc.vector.tensor_tensor(asq[:, hs], at[:, hs], at[:, hs],
                                            MUL)

            def emit_reduce(s):
                rps = rpool.tile([128, 2 * M], f32, tag="rps")
                for gl in range(4):
                    nc.tensor.matmul(
                        rps[32 * gl:32 * gl + 2, 0:M], rw_t[:],
                        at_s[s][:, gl * M:(gl + 1) * M],
                        start=True, stop=True, tile_position=(0, 32 * gl))
                for gl in range(4):
                    nc.tensor.matmul(
                        rps[32 * gl:32 * gl + 2, M:2 * M], rw_t[:],
                        asq_s[s][:, gl * M:(gl + 1) * M],
                        start=True, stop=True, tile_position=(0, 32 * gl))
                rsb = wpool.tile([128, 2 * M], f32, tag="rsb")
                if s < NSETS - 1:
                    nc.vector.tensor_copy(rsb[:], rps[:])
                else:
                    # tail: gv half copies while the gd reduce pack still runs
                    nc.vector.tensor_copy(rsb[:, 0:M], rps[:, 0:M])
                    nc.vector.tensor_copy(rsb[:, M:2 * M], rps[:, M:2 * M])
                if s < NSETS - 1:
                    qs = (nc.sync, nc.gpsimd, nc.sync, nc.gpsimd)
                else:
                    qs = (nc.sync, nc.gpsimd, nc.scalar, nc.sync)
                for gl in range(4):
                    qs[gl].dma_start(out_d[s, gl],
                                     rsb[32 * gl:32 * gl + 2, :])

            for s in range(NSETS):
                emit_main(s, mid=(lambda s=s: emit_reduce(s - 1))
                          if s >= 1 else None)
            emit_reduce(NSETS - 1)

    nc.compile()
    return nc


def _get_program():
    global _PROG
    if _PROG is None:
        _PROG = _build_program()
    return _PROG


# ---------------------------------------------------------------------------
# numpy fallback (general b1; never hit for this model's inputs)
# ---------------------------------------------------------------------------

def _numpy_reference(x, W1, b1, W2, b2, muO):
    def transforms(x):
        outs = []
        for sign in (1.0, -1.0):
            sx = sign * x
            for k in range(4):
                rx = np.rot90(sx, k=k, axes=(1, 2))
                outs.append(rx)
                outs.append(np.flip(rx, axis=1))
        return np.stack(outs)

    idx = (np.arange(L)[:, None] + np.arange(L)[None, :]) % L
    Ftot = np.zeros(B, np.float32)
    for tx in transforms(x):
        fx = _force(tx).reshape(B, V)
        sh = tx[:, idx, :][:, :, :, idx]
        shifts = np.transpose(sh, (1, 3, 0, 2, 4)).reshape(V, B, V)
        z = shifts @ W1 + b1
        h = np.tanh(z)
        gvals = h @ W2 + b2[0]
        grads = ((1.0 - h * h) * W2) @ W1[0]
        Ftot += (grads + gvals * fx.T).sum(axis=0)
    F = Ftot / 16.0
    delta = _computeO(x) - F
    return np.float32(((delta - muO[0]) ** 2).mean())


# ---------------------------------------------------------------------------
# entry point
# ---------------------------------------------------------------------------

def kernel(x, W1, b1, W2, b2, muO):
    x = np.asarray(x, np.float32)
    W1 = np.asarray(W1, np.float32)
    b1 = np.asarray(b1, np.float32)
    W2 = np.asarray(W2, np.float32)
    b2 = np.asarray(b2, np.float32)
    muO = np.asarray(muO, np.float32)

    if np.any(b1 != 0.0):
        return _numpy_reference(x, W1, b1, W2, b2, muO)

    inv_perms, mus = _tables()
    W1flat = W1.reshape(V, H)

    # SH2[(a,c), (i,b)] = x[b, (a+i)%L, c]
    SH2 = np.empty((V, M), np.float32)
    for i in range(L):
        SH2[:, i * B:(i + 1) * B] = np.roll(x, -i, axis=1).reshape(B, V).T
    SH2img = SH2.reshape(L, L, M)

    # per-core inputs: two j-rotations of SH2, shared W1_r images, [W2|CW]
    # w1r cols: (k, r, h); shj cols: (k, jl, m)
    w1r_in = np.empty((2, 128, 8 * H), np.float32)
    for r in range(8):
        img = W1flat[inv_perms[r]].reshape(2, 128, H)
        for k in range(2):
            w1r_in[k, :, r * H:(r + 1) * H] = img[k]
    w1r_in = w1r_in.astype(ml_dtypes.bfloat16)

    CW = (W1flat[0] * W2).astype(np.float32)
    rw_in = np.stack([W2, CW], axis=1).astype(ml_dtypes.bfloat16)

    nc = _get_program()
    from concourse import bass_utils
    in_maps = []
    for cc in range(NCORES):
        shj = np.empty((2, 128, 2 * M), np.float32)
        for jl in range(JPER):
            j = JPER * cc + jl
            rot = np.roll(SH2img, -j, axis=1).reshape(2, 128, M)
            for k in range(2):
                shj[k, :, jl * M:(jl + 1) * M] = rot[k]
        shj = shj.astype(ml_dtypes.bfloat16)
        in_maps.append({"shj0": shj[0], "shj1": shj[1],
                        "w1r0": w1r_in[0], "w1r1": w1r_in[1], "rw": rw_in})
    res = bass_utils.run_bass_kernel_spmd(nc, in_maps,
                                          core_ids=list(range(NCORES)))

    # assemble GV[i,b,j,r], GD[i,b,j,r] from per-core (4, 4, 2, 1024) outputs
    GV = np.empty((L, B, L, 8), np.float32)
    GD = np.empty((L, B, L, 8), np.float32)
    for cc in range(NCORES):
        arr = np.asarray(res.results[cc]["gvgd"])   # (set, gl, 2, 2M)
        for s in range(NSETS):
            for gl in range(4):
                rl, jl = gl >> 1, gl & 1
                r = 2 * s + rl
                j = JPER * cc + jl
                GV[:, :, j, r] = arr[s, gl, 0, 0:M].reshape(L, B)
                GD[:, :, j, r] = arr[s, gl, 1, M:2 * M].reshape(L, B)

    fxo = _force(x).reshape(B, V)
    Csum = float(CW.sum())
    Ftot = np.zeros(B, np.float64)
    for r in range(8):
        gval = GV[:, :, :, r].transpose(0, 2, 1).reshape(V, B)
        gdot = Csum - GD[:, :, :, r].transpose(0, 2, 1).reshape(V, B)
        fxt = fxo[:, mus[r]].T
        Ftot += (gdot + gval * fxt).sum(axis=0)
    F = (Ftot / 8.0).astype(np.float32)

    delta = _computeO(x) - F
    return np.float32(((delta - muO[0]) ** 2).mean())


# revision 23
# speedup vs baseline: 1.0271x; 1.0271x over previous
"""Trainium2 Bass kernel for nn_ControlModel_g (phi^4 lattice control-variate loss).

Math reformulation (validated to fp32 accuracy against the jax reference):

  The reference evaluates, for each of 16 signed lattice symmetries t and all
  V=256 torus translations s, the tiny MLP g (256->128->1) on the transformed+
  shifted configs, plus its input-gradient at site (0,0), combined with the
  phi^4 force into F[b]; loss = mean((computeO(x) - F - muO)^2).

  1. Symmetry transforms move from x onto W1 (g(T_{-s} R x) = g_R(T_{-s'} x)
     with spatially-transformed weights), so all shifted inputs derive from x
     alone and the force/gradient corrections become fixed permutations.
  2. With b1 == 0 (always true for this model), tanh oddness makes the 8
     sign=-1 transforms algebraically redundant -> half the compute.
  3. The column translation j is moved onto the DATA side: instead of 16
     j-rotated copies of W1 per core, each core gets its two j-rotations of
     the shifted-x matrix SHJ_j[(a,c),(i,b)] = x[b,(a+i)%16,(c+j)%16] and the
     8 spatially-permuted W1_r weight images are shared across j. Device work
     per core: 16 group matmuls Z_{r,jl} = W1_r^T @ SHJ_jl (f32 accum over
     the 256-site contraction in 2 psum passes), tanh (scalar ACT), square
     (DVE), and two 1-row PE reductions per group with a shared [W2|CW]
     stationary (tile_position column strips, 4 groups per PSUM set).
  4. All operands are bf16 (error budget validated: ~6e-4 rel on the loss vs
     2e-2 tolerance); psum accumulation stays fp32.
  5. Sharding: data-parallel over the j columns - core cc takes j in
     {2cc, 2cc+1}. No collectives; the final O(B*V) combine (force
     permutations, computeO, loss) is host-side numpy.
"""

import numpy as np
import ml_dtypes

L = 16
Y = 4
KAPPA = 0.25
LAM = 0.5
B = 32
V = L * L          # 256
H = 128
NCORES = 8
JPER = L // NCORES         # j values per core = 2
M = L * B                  # 512 columns (i, b) per group
NSETS = 4                  # r-pairs; each set = 4 groups (rl, jl)

# ---------------------------------------------------------------------------
# host-side lattice helpers
# ---------------------------------------------------------------------------

def _force(phi):
    nbr = (np.roll(phi, 1, 1) + np.roll(phi, -1, 1)
           + np.roll(phi, 1, 2) + np.roll(phi, -1, 2))
    return 2.0 * KAPPA * nbr - 2.0 * phi - 4.0 * LAM * phi * (phi * phi - 1.0)


def _computeO(x):
    x0 = x.mean(axis=1)
    x0 = x0 - x0.mean(axis=0, keepdims=True)
    return (x0 * np.roll(x0, -Y, axis=1)).mean(axis=1)


def _spatial_ops():
    ops = []
    for k in range(4):
        ops.append(lambda y, k=k: np.rot90(y, k=k, axes=(0, 1)))
        ops.append(lambda y, k=k: np.flip(np.rot90(y, k=k, axes=(0, 1)), axis=0))
    return ops


def _op_tables():
    """Per spatial op r: inverse site permutation (for W1) and the force
    permutation mu_r[s] = pi_r(rho_r^{-1}(s))."""
    ops = _spatial_ops()
    IDX = np.arange(V).reshape(L, L)
    inv_perms, mus = [], []
    for op in ops:
        pi = op(IDX).reshape(-1)
        inv = np.empty(V, np.int64)
        inv[pi] = np.arange(V)
        inv_perms.append(inv)
        rho = np.empty(V, np.int64)
        opIDX = op(IDX)
        for i in range(L):
            for j in range(L):
                shifted = np.roll(np.roll(opIDX, -i, 0), -j, 1).reshape(V)
                rho[i * L + j] = shifted[inv][0]
        rho_inv = np.empty(V, np.int64)
        rho_inv[rho] = np.arange(V)
        mus.append(pi[rho_inv])
    return inv_perms, mus


_TABLES = None

def _tables():
    global _TABLES
    if _TABLES is None:
        _TABLES = _op_tables()
    return _TABLES


# ---------------------------------------------------------------------------
# device program (built once, cached)
# ---------------------------------------------------------------------------

_PROG = None

def _build_program():
    import concourse.bass as bass
    import concourse.tile as tile
    from concourse import bacc, mybir

    f32 = mybir.dt.float32
    bf16 = mybir.dt.bfloat16
    MUL = mybir.AluOpType.mult

    nc = bacc.Bacc("TRN2", target_bir_lowering=False, debug=False,
                   num_devices=NCORES)

    # packed inputs: few big fully-contiguous DMAs (each dma_start costs
    # ~600ns of queue time; column-sliced HBM reads run at half bandwidth)
    # shj_k cols: (jl, m); w1r_k cols: (r, h)
    shj_ds = [nc.dram_tensor(f"shj{k}", (128, 2 * M), bf16,
                             kind="ExternalInput") for k in range(2)]
    w1r_ds = [nc.dram_tensor(f"w1r{k}", (128, 8 * H), bf16,
                             kind="ExternalInput") for k in range(2)]
    rw_d = nc.dram_tensor("rw", (128, 2), bf16, kind="ExternalInput")
    out_d = nc.dram_tensor("gvgd", (NSETS, 4, 2, 2 * M), f32,
                           kind="ExternalOutput")

    NWARM = 9

    with tile.TileContext(nc) as tc:
        with (
            tc.tile_pool(name="consts", bufs=1) as cpool,
            tc.tile_pool(name="zp", bufs=3, space=bass.MemorySpace.PSUM) as zpool,
            tc.tile_pool(name="rp", bufs=1, space=bass.MemorySpace.PSUM) as rpool,
            tc.tile_pool(name="work", bufs=2) as wpool,
        ):
            # ---- PE warmup: keep the HAM clock-gate busy during input DMA -
            if NWARM:
                ws = cpool.tile([128, M + 128], bf16, tag="ws")
                nc.vector.memset(ws[:], 0.0)
                wz = zpool.tile([128, 2 * M], f32, tag="zt")
                for _ in range(NWARM):
                    nc.tensor.matmul(wz[:, 0:M], ws[:, 0:128], ws[:, 128:],
                                     start=True, stop=True)

            # ---- input DMAs: 5 contiguous transfers on two idle queues ----
            # shj first (gates the first matmuls); rw last (needed ~10us in)
            shj_t, w1r_t = [], []
            for k in range(2):
                sht = cpool.tile([128, 2 * M], bf16, tag=f"shj{k}")
                nc.sync.dma_start(sht[:], shj_ds[k][:])
                shj_t.append(sht)
                w1t = cpool.tile([128, 8 * H], bf16, tag=f"w1r{k}")
                nc.gpsimd.dma_start(w1t[:], w1r_ds[k][:])
                w1r_t.append(w1t)
            rw_t = cpool.tile([128, 2], bf16, tag="rw")
            nc.sync.dma_start(rw_t[:], rw_d[:])

            def shj_blk(jl, k):
                return shj_t[k][:, jl * M:(jl + 1) * M]

            def w1r_blk(r, k):
                return w1r_t[k][:, r * H:(r + 1) * H]

            # ---- pipelined compute ---------------------------------------
            at_s = [None] * NSETS
            asq_s = [None] * NSETS

            def emit_main(s, mid=None):
                at = wpool.tile([128, 4 * M], bf16, tag="at")
                asq = wpool.tile([128, 4 * M], bf16, tag="asq")
                at_s[s], asq_s[s] = at, asq
                for rl in range(2):
                    if rl == 1 and mid is not None:
                        mid()
                    r = 2 * s + rl
                    zt = zpool.tile([128, 2 * M], f32, tag="zt")
                    for k in range(2):
                        for jl in range(2):
                            nc.tensor.matmul(
                                zt[:, jl * M:(jl + 1) * M],
                                w1r_blk(r, k),
                                shj_blk(jl, k),
                                start=(k == 0),
                                stop=(k == 1),
                            )
                    hs = slice(rl * 2 * M, (rl + 1) * 2 * M)
                    nc.scalar.activation(at[:, hs], zt[:],
                                         mybir.ActivationFunctionType.Tanh)
                    nc.vector.tensor_tensor(asq[:, hs], at[:, hs], at[:, hs],
                                            MUL)

            def emit_reduce(s):
                rps = rpool.tile([128, 2 * M], f32, tag="rps")
                for gl in range(4):
                    nc.tensor.matmul(
                        rps[32 * gl:32 * gl + 2, 0:M], rw_t[:],
                        at_s[s][:, gl * M:(gl + 1) * M],
                        start=True, stop=True, tile_position=(0, 32 * gl))
                for gl in range(4):
                    nc.tensor.matmul(
                        rps[32 * gl:32 * gl + 2, M:2 * M], rw_t[:],
                        asq_s[s][:, gl * M:(gl + 1) * M],
                        start=True, stop=True, tile_position=(0, 32 * gl))
                rsb = wpool.tile([128, 2 * M], f32, tag="rsb")
                if s < NSETS - 1:
                    nc.vector.tensor_copy(rsb[:], rps[:])
                else:
                    # tail: gv half copies while the gd reduce pack still runs
                    nc.vector.tensor_copy(rsb[:, 0:M], rps[:, 0:M])
                    nc.vector.tensor_copy(rsb[:, M:2 * M], rps[:, M:2 * M])
                if s < NSETS - 1:
                    qs = (nc.sync, nc.gpsimd, nc.sync, nc.gpsimd)
                else:
                    qs = (nc.sync, nc.gpsimd, nc.scalar, nc.sync)
                for gl in range(4):
                    qs[gl].dma_start(out_d[s, gl],
                                     rsb[32 * gl:32 * gl + 2, :])

            for s in range(NSETS):
                emit_main(s)
                if s >= 1:
                    emit_reduce(s - 1)
            emit_reduce(NSETS - 1)

    nc.compile()
    return nc


def _get_program():
    global _PROG
    if _PROG is None:
        _PROG = _build_program()
    return _PROG


# ---------------------------------------------------------------------------
# numpy fallback (general b1; never hit for this model's inputs)
# ---------------------------------------------------------------------------

def _numpy_reference(x, W1, b1, W2, b2, muO):
    def transforms(x):
        outs = []
        for sign in (1.0, -1.0):
            sx = sign * x
            for k in range(4):
                rx = np.rot90(sx, k=k, axes=(1, 2))
                outs.append(rx)
                outs.append(np.flip(rx, axis=1))
        return np.stack(outs)

    idx = (np.arange(L)[:, None] + np.arange(L)[None, :]) % L
    Ftot = np.zeros(B, np.float32)
    for tx in transforms(x):
        fx = _force(tx).reshape(B, V)
        sh = tx[:, idx, :][:, :, :, idx]
        shifts = np.transpose(sh, (1, 3, 0, 2, 4)).reshape(V, B, V)
        z = shifts @ W1 + b1
        h = np.tanh(z)
        gvals = h @ W2 + b2[0]
        grads = ((1.0 - h * h) * W2) @ W1[0]
        Ftot += (grads + gvals * fx.T).sum(axis=0)
    F = Ftot / 16.0
    delta = _computeO(x) - F
    return np.float32(((delta - muO[0]) ** 2).mean())


# ---------------------------------------------------------------------------
# entry point
# ---------------------------------------------------------------------------

def kernel(x, W1, b1, W2, b2, muO):
    x = np.asarray(x, np.float32)
    W1 = np.asarray(W1, np.float32)
    b1 = np.asarray(b1, np.float32)
    W2 = np.asarray(W2, np.float32)
    b2 = np.asarray(b2, np.float32)
    muO = np.asarray(muO, np.float32)

    if np.any(b1 != 0.0):
        return _numpy_reference(x, W1, b1, W2, b2, muO)

    inv_perms, mus = _tables()
    W1flat = W1.reshape(V, H)

    # SH2[(a,c), (i,b)] = x[b, (a+i)%L, c]
    SH2 = np.empty((V, M), np.float32)
    for i in range(L):
        SH2[:, i * B:(i + 1) * B] = np.roll(x, -i, axis=1).reshape(B, V).T
    SH2img = SH2.reshape(L, L, M)

    # per-core inputs: two j-rotations of SH2, shared W1_r images, [W2|CW]
    # w1r cols: (k, r, h); shj cols: (k, jl, m)
    w1r_in = np.empty((2, 128, 8 * H), np.float32)
    for r in range(8):
        img = W1flat[inv_perms[r]].reshape(2, 128, H)
        for k in range(2):
            w1r_in[k, :, r * H:(r + 1) * H] = img[k]
    w1r_in = w1r_in.astype(ml_dtypes.bfloat16)

    CW = (W1flat[0] * W2).astype(np.float32)
    rw_in = np.stack([W2, CW], axis=1).astype(ml_dtypes.bfloat16)

    nc = _get_program()
    from concourse import bass_utils
    in_maps = []
    for cc in range(NCORES):
        shj = np.empty((2, 128, 2 * M), np.float32)
        for jl in range(JPER):
            j = JPER * cc + jl
            rot = np.roll(SH2img, -j, axis=1).reshape(2, 128, M)
            for k in range(2):
                shj[k, :, jl * M:(jl + 1) * M] = rot[k]
        shj = shj.astype(ml_dtypes.bfloat16)
        in_maps.append({"shj0": shj[0], "shj1": shj[1],
                        "w1r0": w1r_in[0], "w1r1": w1r_in[1], "rw": rw_in})
    res = bass_utils.run_bass_kernel_spmd(nc, in_maps,
                                          core_ids=list(range(NCORES)))

    # assemble GV[i,b,j,r], GD[i,b,j,r] from per-core (4, 4, 2, 1024) outputs
    GV = np.empty((L, B, L, 8), np.float32)
    GD = np.empty((L, B, L, 8), np.float32)
    for cc in range(NCORES):
        arr = np.asarray(res.results[cc]["gvgd"])   # (set, gl, 2, 2M)
        for s in range(NSETS):
            for gl in range(4):
                rl, jl = gl >> 1, gl & 1
                r = 2 * s + rl
                j = JPER * cc + jl
                GV[:, :, j, r] = arr[s, gl, 0, 0:M].reshape(L, B)
                GD[:, :, j, r] = arr[s, gl, 1, M:2 * M].reshape(L, B)

    fxo = _force(x).reshape(B, V)
    Csum = float(CW.sum())
    Ftot = np.zeros(B, np.float64)
    for r in range(8):
        gval = GV[:, :, :, r].transpose(0, 2, 1).reshape(V, B)
        gdot = Csum - GD[:, :, :, r].transpose(0, 2, 1).reshape(V, B)
        fxt = fxo[:, mus[r]].T
        Ftot += (gdot + gval * fxt).sum(axis=0)
    F = (Ftot / 8.0).astype(np.float32)

    delta = _computeO(x) - F
    return np.float32(((delta - muO[0]) ** 2).mean())
